# revision 2
# baseline (speedup 1.0000x reference)
"""NF4-style 4-bit quantized linear: out = x @ dequant(w).T on 8 TRN2 NeuronCores.

Column-parallel sharding: core c owns output features [c*512, (c+1)*512) and the
corresponding contiguous slices of the packed weight + quant state arrays. x is
replicated. Each core:
  1. dequantizes its 512x4096 weight slice on-chip (DVE) into fp16,
  2. round-trips it through DRAM with an xbar transpose DMA to get wT
     [k-partition, outf] layout,
  3. streams x through xbar transpose DMAs ([token, k] -> [k, token]) and runs
     the fp16 matmul on the PE array, accumulating in PSUM over 32 k-tiles.
Host gathers the per-core [8192, 512] outputs with a concat along axis 1.
"""
import numpy as np

import concourse.bass as bass
import concourse.mybir as mybir
import concourse.tile as tile
from concourse import bacc
from concourse.bass_utils import run_bass_kernel_spmd

F16 = mybir.dt.float16
F32 = mybir.dt.float32
I32 = mybir.dt.int32
Alu = mybir.AluOpType

P = 128
TOKENS = 8192
IN_F = 4096
OUT_F = 4096
N_CORES = 8
O_C = OUT_F // N_CORES          # 512 out features per core
KT = IN_F // P                  # 32 k-tiles
BPR = IN_F // 2                 # 2048 packed bytes per weight row
NB_O = O_C // P                 # 4 o-tiles of 128 rows
TB = 512                        # token block
BC = 2                          # byte-column chunks per o-tile (1024 bytes each)
BCW = BPR // BC                 # 1024


def _build(tokens=TOKENS):
    ntb = tokens // TB
    nc = bacc.Bacc("TRN2", target_bir_lowering=False, debug=False,
                   enable_asserts=False)

    x = nc.dram_tensor("x", [tokens, IN_F], F16, kind="ExternalInput").ap()
    qw = nc.dram_tensor("qw", [O_C, BPR], I32, kind="ExternalInput").ap()
    qam = nc.dram_tensor("qam", [O_C, 64], I32, kind="ExternalInput").ap()
    qcode = nc.dram_tensor("qcode", [O_C, 64], F32, kind="ExternalInput").ap()
    qoff = nc.dram_tensor("qoff", [O_C, 64], F32, kind="ExternalInput").ap()
    am2 = nc.dram_tensor("am2", [O_C, 16], F32, kind="ExternalInput").ap()
    c2 = nc.dram_tensor("c2", [O_C, 16], F32, kind="ExternalInput").ap()
    out = nc.dram_tensor("out", [tokens, O_C], F16, kind="ExternalOutput").ap()

    with tile.TileContext(nc) as tc:
        with tc.tile_pool(name="wt_pool", bufs=1) as wt_pool, \
             tc.tile_pool(name="wdram", bufs=1, space="DRAM") as wdram:
            wt = wt_pool.tile([P, KT, O_C], F16, name="wt")
            w_scratch = wdram.tile([O_C, IN_F], F16, name="w_scratch")

            # ---- Phase A: dequantize into natural [outf, k] layout ----
            with tc.tile_pool(name="dq", bufs=2) as dq, \
                 tc.tile_pool(name="dqw", bufs=2) as dqw:
                for ot in range(NB_O):
                    rs = slice(ot * P, (ot + 1) * P)
                    am_f = dq.tile([P, 64], F32, name="am_f")
                    nc.gpsimd.dma_start(am_f, qam[rs, :])   # int32 -> f32 cast
                    cd_t = dq.tile([P, 64], F32, name="cd_t")
                    nc.sync.dma_start(cd_t, qcode[rs, :])
                    of_t = dq.tile([P, 64], F32, name="of_t")
                    nc.sync.dma_start(of_t, qoff[rs, :])
                    am2_t = dq.tile([P, 16], F32, name="am2_t")
                    nc.sync.dma_start(am2_t, am2[rs, :])
                    c2_t = dq.tile([P, 16], F32, name="c2_t")
                    nc.sync.dma_start(c2_t, c2[rs, :])

                    rc = dq.tile([P, 64], F32, name="rc")
                    nc.vector.reciprocal(rc, cd_t)
                    s1 = dq.tile([P, 64], F32, name="s1")
                    nc.vector.tensor_tensor(s1, am_f, rc, Alu.mult)
                    rc2 = dq.tile([P, 16], F32, name="rc2")
                    nc.vector.reciprocal(rc2, c2_t)
                    s2 = dq.tile([P, 16], F32, name="s2")
                    nc.vector.tensor_tensor(s2, am2_t, rc2, Alu.mult)
                    S = dq.tile([P, 64], F32, name="S")
                    nc.vector.tensor_tensor(
                        S, s1, s2.unsqueeze(2).broadcast_to([P, 16, 4]), Alu.mult)
                    offS = dq.tile([P, 64], F32, name="offS")
                    nc.vector.tensor_tensor(offS, of_t, S, Alu.mult)

                    w_nat = dqw.tile([P, BC, 2 * BCW], F16, name="w_nat")
                    for bc in range(BC):
                        cs = slice(bc * BCW, (bc + 1) * BCW)
                        qt = dq.tile([P, BCW], I32, name="qt")
                        nc.sync.dma_start(qt, qw[rs, cs])
                        lo = dq.tile([P, BCW], I32, name="lo")
                        nc.vector.tensor_scalar(lo, qt, 15, None, Alu.bitwise_and)
                        hi = dq.tile([P, BCW], I32, name="hi")
                        nc.vector.tensor_scalar(hi, qt, 4, None,
                                                Alu.logical_shift_right)
                        nblk = BCW // 32  # 32 blocks per chunk
                        S_b = S[:, bc * nblk:(bc + 1) * nblk] \
                            .unsqueeze(2).broadcast_to([P, nblk, 32])
                        offS_b = offS[:, bc * nblk:(bc + 1) * nblk] \
                            .unsqueeze(2).broadcast_to([P, nblk, 32])
                        we = dq.tile([P, BCW], F32, name="we")
                        nc.vector.tensor_tensor(we, lo, S_b, Alu.mult)
                        wo = dq.tile([P, BCW], F32, name="wo")
                        nc.vector.tensor_tensor(wo, hi, S_b, Alu.mult)
                        nc.vector.tensor_tensor(
                            w_nat[:, bc, 0::2], we, offS_b, Alu.subtract)
                        nc.vector.tensor_tensor(
                            w_nat[:, bc, 1::2], wo, offS_b, Alu.subtract)
                    nc.sync.dma_start(
                        w_scratch[rs, :],
                        w_nat.rearrange("p a b -> p (a b)"))

            # ---- Phase B: transpose w via xbar DMA: wt[p, kk, o] = w[o, kk*128+p]
            nc.sync.dma_start(out=wt, in_=w_scratch[:, :], transpose=True)

            # ---- Phase C: stream x, matmul ----
            with tc.tile_pool(name="xt_pool", bufs=2) as xt_pool, \
                 tc.tile_pool(name="ps_pool", bufs=4, space="PSUM") as ps_pool, \
                 tc.tile_pool(name="ob_pool", bufs=4) as ob_pool:
                for tb in range(ntb):
                    xt = xt_pool.tile([P, KT, TB], F16, name="xt")
                    nc.sync.dma_start(
                        out=xt, in_=x[tb * TB:(tb + 1) * TB, :], transpose=True)
                    for st in range(TB // P):
                        ps = ps_pool.tile([P, O_C], F32, name="ps")
                        for kk in range(KT):
                            nc.tensor.matmul(
                                ps,
                                xt[:, kk, st * P:(st + 1) * P],
                                wt[:, kk, :],
                                start=(kk == 0),
                                stop=(kk == KT - 1),
                            )
                        ob = ob_pool.tile([P, O_C], F16, name="ob")
                        nc.vector.tensor_copy(ob, ps)
                        nc.sync.dma_start(
                            out[tb * TB + st * P: tb * TB + (st + 1) * P, :], ob)

    nc.compile()
    return nc


_NC_CACHE = {}


def _get_nc(tokens=TOKENS):
    if tokens not in _NC_CACHE:
        _NC_CACHE[tokens] = _build(tokens)
    return _NC_CACHE[tokens]


def _shard(inputs):
    x = np.ascontiguousarray(np.asarray(inputs["x"], dtype=np.float16))
    qw = np.asarray(inputs["quantized_weight"], dtype=np.int32)
    qam = np.asarray(inputs["quant_absmax"], dtype=np.int32)
    qcode = np.asarray(inputs["quant_code"], dtype=np.float32)
    qoff = np.asarray(inputs["quant_offset"], dtype=np.float32)
    am2 = np.asarray(inputs["state2_absmax"], dtype=np.float32)
    c2 = np.asarray(inputs["state2_code"], dtype=np.float32)

    pb = O_C * BPR        # packed bytes per core
    nb1 = O_C * 64        # primary blocks per core
    nb2 = O_C * 16        # secondary blocks per core
    in_maps = []
    for c in range(N_CORES):
        in_maps.append({
            "x": x,
            "qw": np.ascontiguousarray(
                qw[c * pb:(c + 1) * pb].reshape(O_C, BPR)),
            "qam": np.ascontiguousarray(
                qam[c * nb1:(c + 1) * nb1].reshape(O_C, 64)),
            "qcode": np.ascontiguousarray(
                qcode[c * nb1:(c + 1) * nb1].reshape(O_C, 64)),
            "qoff": np.ascontiguousarray(
                qoff[c * nb1:(c + 1) * nb1].reshape(O_C, 64)),
            "am2": np.ascontiguousarray(
                am2[c * nb2:(c + 1) * nb2].reshape(O_C, 16)),
            "c2": np.ascontiguousarray(
                c2[c * nb2:(c + 1) * nb2].reshape(O_C, 16)),
        })
    return in_maps


def _run(inputs, trace=False, trace_cores=None):
    nc = _get_nc()
    in_maps = _shard(inputs)
    res = run_bass_kernel_spmd(
        nc, in_maps, list(range(N_CORES)), trace=trace,
        trace_cores=trace_cores)
    out = np.concatenate([r["out"] for r in res.results], axis=1)
    return out, res


def kernel(**inputs) -> np.ndarray:
    out, _ = _run(inputs, trace=False)
    return out
